# revision 4
# baseline (speedup 1.0000x reference)
"""Multi-head attention (B=2, S=2048, D=1024, H=16) on 8 TRN2 NeuronCores.

Sharding: core c handles batch b = c // 4 and heads 4*(c%4) .. 4*(c%4)+3.
Each core computes its 4 heads' Q/K/V projections (column slices of the
weights), head-local attention, and a partial output projection (row slice
of Wo). Host sums the 4 partials per batch. No cross-device collectives.

All matmuls run in fp32r (full PE rate, ~1e-4 rounding). The kernel works
in a transposed layout throughout: inputs are fed as x^T [D, S] so
projections produce Q^T/K^T directly, scores are computed as S^T = K_h^T.T
@ Q_h^T (shape [k, q]), the softmax denominator falls out of the P@V matmul
via an extra ones-column on V, and the output projection produces the
partial out^T [D, S] which the host transposes back.
"""

import numpy as np

import concourse.bacc as bacc
import concourse.mybir as mybir
import concourse.tile as tile
from concourse import masks
from concourse.bass_utils import run_bass_kernel_spmd

B, S, D, H = 2, 2048, 1024, 16
HD = D // H            # 64
N_CORES = 8
HPC = H // (N_CORES // B)   # heads per core = 4
HG = HPC * HD               # head-group width = 256

F32 = mybir.dt.float32
F32R = mybir.dt.float32r
F16 = mybir.dt.float16
AF = mybir.ActivationFunctionType
P = 128

_CACHE = {}


def _build():
    nc = bacc.Bacc("TRN2", target_bir_lowering=False, debug=False,
                   num_devices=N_CORES)

    qt_d = nc.dram_tensor("qt", [D, S], F16, kind="ExternalInput")
    kt_d = nc.dram_tensor("kt", [D, S], F16, kind="ExternalInput")
    vt_d = nc.dram_tensor("vt", [D, S], F16, kind="ExternalInput")
    wq_d = nc.dram_tensor("wq", [D, HG], F16, kind="ExternalInput")
    wk_d = nc.dram_tensor("wk", [D, HG], F16, kind="ExternalInput")
    wv_d = nc.dram_tensor("wv", [D, HG], F16, kind="ExternalInput")
    wo_d = nc.dram_tensor("wo", [HG, D], F16, kind="ExternalInput")
    bq_d = nc.dram_tensor("bq", [HG, 1], F32, kind="ExternalInput")
    bk_d = nc.dram_tensor("bk", [HG, 1], F32, kind="ExternalInput")
    bv_d = nc.dram_tensor("bv", [HG, 1], F32, kind="ExternalInput")
    out_d = nc.dram_tensor("outT", [D, S], F32, kind="ExternalOutput")

    NDK = D // P     # 8 contraction tiles for projections
    NM = HG // P     # 2 row tiles for Q^T/K^T/V^T
    NKT = S // P     # 16 key tiles
    QC1 = 512        # phase-1 free-dim chunk
    QC2 = 1024       # phase-2/3 q chunk

    with tile.TileContext(nc) as tc:
        with tc.tile_pool(name="persist", bufs=1) as pp:
            qt_sb = [pp.tile([P, S], F16, tag=f"qt{m}", name=f"qt_sb{m}") for m in range(NM)]
            kt_sb = [pp.tile([P, S], F16, tag=f"kt{m}", name=f"kt_sb{m}") for m in range(NM)]
            at_sb = [pp.tile([P, S], F16, tag=f"at{m}", name=f"at_sb{m}") for m in range(NM)]
            vb = pp.tile([P, NKT, HPC, 2 * HD], F16)
            wq_sb = pp.tile([P, NDK, HG], F16, tag="wq")
            wk_sb = pp.tile([P, NDK, HG], F16, tag="wk")
            wv_sb = pp.tile([P, NDK, HG], F16, tag="wv")
            wo_sb = pp.tile([P, NM, D], F16, tag="wo")
            bq_sb = pp.tile([P, NM], F32, tag="bq")
            bk_sb = pp.tile([P, NM], F32, tag="bk")
            bv_sb = pp.tile([P, NM], F32, tag="bv")
            ident = pp.tile([P, P], F16)

            nc.sync.dma_start(wq_sb[:], wq_d[:].rearrange("(a p) n -> p a n", p=P))
            nc.sync.dma_start(wk_sb[:], wk_d[:].rearrange("(a p) n -> p a n", p=P))
            nc.sync.dma_start(wv_sb[:], wv_d[:].rearrange("(a p) n -> p a n", p=P))
            nc.sync.dma_start(wo_sb[:], wo_d[:].rearrange("(a p) n -> p a n", p=P))
            nc.sync.dma_start(bq_sb[:], bq_d[:].rearrange("(a p) o -> p (a o)", p=P))
            nc.sync.dma_start(bk_sb[:], bk_d[:].rearrange("(a p) o -> p (a o)", p=P))
            nc.sync.dma_start(bv_sb[:], bv_d[:].rearrange("(a p) o -> p (a o)", p=P))
            masks.make_identity(nc, ident[:])

            with tc.tile_pool(name="vtp", bufs=1) as vtp:
                vt_sb = [vtp.tile([P, S], F16, tag=f"vt{m}", name=f"vt_sb{m}") for m in range(NM)]

                # ---- Phase 1: projections Q^T, K^T, V^T = W.T @ x^T ----
                with (
                    tc.tile_pool(name="p1in", bufs=3) as p1in,
                    tc.tile_pool(name="p1ps", bufs=1, space="PSUM") as p1ps,
                ):
                    for qc in range(S // QC1):
                        sl = slice(qc * QC1, (qc + 1) * QC1)
                        ps_q = [p1ps.tile([P, QC1], F32, tag=f"pq{m}", name=f"ps_q{m}") for m in range(NM)]
                        ps_k = [p1ps.tile([P, QC1], F32, tag=f"pk{m}", name=f"ps_k{m}") for m in range(NM)]
                        ps_v = [p1ps.tile([P, QC1], F32, tag=f"pv{m}", name=f"ps_v{m}") for m in range(NM)]
                        for kt in range(NDK):
                            rows = slice(kt * P, (kt + 1) * P)
                            xq = p1in.tile([P, QC1], F16, tag="xq")
                            xk = p1in.tile([P, QC1], F16, tag="xk")
                            xv = p1in.tile([P, QC1], F16, tag="xv")
                            nc.sync.dma_start(xq[:], qt_d[rows, sl])
                            nc.sync.dma_start(xk[:], kt_d[rows, sl])
                            nc.sync.dma_start(xv[:], vt_d[rows, sl])
                            st, sp = kt == 0, kt == NDK - 1
                            for m in range(NM):
                                cols = slice(m * P, (m + 1) * P)
                                nc.tensor.matmul(ps_q[m][:], wq_sb[:, kt, cols],
                                                 xq[:], start=st, stop=sp)
                                nc.tensor.matmul(ps_k[m][:], wk_sb[:, kt, cols],
                                                 xk[:], start=st, stop=sp)
                                nc.tensor.matmul(ps_v[m][:], wv_sb[:, kt, cols],
                                                 xv[:], start=st, stop=sp)
                        for m in range(NM):
                            nc.vector.tensor_scalar_add(qt_sb[m][:, sl], ps_q[m][:],
                                                        bq_sb[:, m:m + 1])
                            nc.vector.tensor_scalar_add(kt_sb[m][:, sl], ps_k[m][:],
                                                        bk_sb[:, m:m + 1])
                            nc.vector.tensor_scalar_add(vt_sb[m][:, sl], ps_v[m][:],
                                                        bv_sb[:, m:m + 1])

                # ---- Phase 1.5: V^T -> V tiles (+ ones column) ----
                with (
                    tc.tile_pool(name="trp", bufs=4, space="PSUM") as trp,
                    tc.tile_pool(name="trs", bufs=1) as trs,
                ):
                    ones_t = trs.tile([P, NKT * HPC * HD], F32)
                    nc.gpsimd.memset(ones_t[:], 1.0)
                    nc.vector.tensor_copy(
                        vb[:, :, :, HD:2 * HD],
                        ones_t[:].rearrange("p (a b c) -> p a b c", a=NKT, b=HPC))
                    for m in range(NM):
                        for f in range(NKT):
                            ptr = trp.tile([P, P], F16, tag="tr")
                            nc.tensor.transpose(ptr[:], vt_sb[m][:, f * P:(f + 1) * P],
                                                ident[:])
                            nc.vector.tensor_copy(vb[:, f, 2 * m, 0:HD], ptr[:, 0:HD])
                            nc.vector.tensor_copy(vb[:, f, 2 * m + 1, 0:HD],
                                                  ptr[:, HD:2 * HD])

            # ---- Phase 2 + 3: attention + output projection ----
            with (
                tc.tile_pool(name="p2ps", bufs=2, space="PSUM") as p2ps,
                tc.tile_pool(name="p2po", bufs=2, space="PSUM") as p2po,
                tc.tile_pool(name="p2sb", bufs=3) as p2sb,
                tc.tile_pool(name="p3sb", bufs=3) as p3sb,
            ):
                for qc in range(S // QC2):
                    qsl = slice(qc * QC2, (qc + 1) * QC2)
                    for h in range(HPC):
                        mh, r0 = h // 2, (h % 2) * HD
                        hr = slice(r0, r0 + HD)
                        po = p2po.tile([P, QC2], F32, tag="po")
                        for kt in range(NKT):
                            kcols = slice(kt * P, (kt + 1) * P)
                            ps = p2ps.tile([P, QC2], F32, tag="ps")
                            for j in range(QC2 // 512):
                                jsl = slice(j * 512, (j + 1) * 512)
                                qjsl = slice(qc * QC2 + j * 512,
                                             qc * QC2 + (j + 1) * 512)
                                nc.tensor.matmul(ps[:, jsl], kt_sb[mh][hr, kcols],
                                                 qt_sb[mh][hr, qjsl],
                                                 start=True, stop=True)
                            pt = p2sb.tile([P, QC2], F16, tag="pt")
                            nc.scalar.activation(pt[:], ps[:], AF.Exp)
                            for j in range(QC2 // 512):
                                jsl = slice(j * 512, (j + 1) * 512)
                                nc.tensor.matmul(po[:, jsl], vb[:, kt, h, :],
                                                 pt[:, jsl], start=(kt == 0),
                                                 stop=(kt == NKT - 1))
                        # normalize: rows 0:HD / rows HD:2HD (replicated row-sums)
                        rc = p2sb.tile([HD, QC2], F32, tag="rc")
                        nc.vector.reciprocal(rc[:], po[HD:2 * HD, :])
                        nc.vector.tensor_tensor(at_sb[mh][hr, qsl], po[0:HD, :],
                                                rc[:], mybir.AluOpType.mult)
                    # output projection for this q chunk: out^T = wo.T @ attn^T
                    for m in range(NDK):
                        cols = slice(m * P, (m + 1) * P)
                        pp3 = p2po.tile([P, QC2], F32, tag="po")
                        for kt2 in range(NM):
                            for j in range(QC2 // 512):
                                jsl = slice(j * 512, (j + 1) * 512)
                                qjsl = slice(qc * QC2 + j * 512,
                                             qc * QC2 + (j + 1) * 512)
                                nc.tensor.matmul(pp3[:, jsl], wo_sb[:, kt2, cols],
                                                 at_sb[kt2][:, qjsl],
                                                 start=(kt2 == 0), stop=(kt2 == NM - 1))
                        ot = p3sb.tile([P, QC2], F32, tag="ot")
                        nc.vector.tensor_copy(ot[:], pp3[:])
                        nc.sync.dma_start(out_d[cols, qsl], ot[:])

    nc.compile()
    return nc


def kernel(query, key, value, Wq, bq, Wk, bk, Wv, bv, Wo, bo):
    if "nc" not in _CACHE:
        _CACHE["nc"] = _build()
    nc = _CACHE["nc"]

    scale = np.float32(1.0 / np.sqrt(HD))
    xt = {}
    for b in range(B):
        xt[("q", b)] = np.ascontiguousarray(query[b].T).astype(np.float16)
        xt[("k", b)] = np.ascontiguousarray(key[b].T).astype(np.float16)
        xt[("v", b)] = np.ascontiguousarray(value[b].T).astype(np.float16)

    in_maps = []
    for c in range(N_CORES):
        b, g = c // (N_CORES // B), c % (N_CORES // B)
        cols = slice(g * HG, (g + 1) * HG)
        in_maps.append({
            "qt": xt[("q", b)],
            "kt": xt[("k", b)],
            "vt": xt[("v", b)],
            "wq": (np.ascontiguousarray(Wq[:, cols]) * scale).astype(np.float16),
            "wk": np.ascontiguousarray(Wk[:, cols]).astype(np.float16),
            "wv": np.ascontiguousarray(Wv[:, cols]).astype(np.float16),
            "wo": np.ascontiguousarray(Wo[cols, :]).astype(np.float16),
            "bq": (bq[cols] * scale).reshape(HG, 1).astype(np.float32),
            "bk": bk[cols].reshape(HG, 1).astype(np.float32),
            "bv": bv[cols].reshape(HG, 1).astype(np.float32),
        })

    global _last_in_maps
    _last_in_maps = in_maps
    res = run_bass_kernel_spmd(nc, in_maps, list(range(N_CORES)))

    out = np.zeros((B, S, D), dtype=np.float32)
    for c in range(N_CORES):
        b = c // (N_CORES // B)
        out[b] += res.results[c]["outT"].T
    out += bo.astype(np.float32)
    return out


# revision 7
# speedup vs baseline: 1.6454x; 1.6454x over previous
"""Multi-head attention (B=2, S=2048, D=1024, H=16) on 8 TRN2 NeuronCores.

Sharding: core c handles batch b = c // 4 and heads 4*(c%4) .. 4*(c%4)+3.
Each core computes its 4 heads' Q/K/V projections (column slices of the
weights), head-local attention, and a partial output projection (row slice
of Wo). Host sums the 4 partials per batch. No cross-device collectives.

Layout: everything transposed. Inputs arrive as x^T [D, S] fp16, so the
projections produce Q^T/K^T directly (lhsT = weight slice, rhs = x^T).
Scores are computed per head as S^T = K_h^T.T @ Q_h^T in [k, q] layout, exp
runs on ScalarE straight out of PSUM into fp16 staging tiles, and the P@V
matmul uses V tiles augmented with a 64-wide ones block so the softmax
denominators appear as 64 replicated rows of the same PSUM accumulator
(full-width DVE reciprocal, no partition broadcast). The output projection
emits the partial out^T [D, S] in fp32; the host transposes and sums.

The attention streams for head t and head t-1 are interleaved at k-tile
granularity so the tensor engine never waits on the (slower) ScalarE exp
stream: per k-tile the PE does 4 matmuls (~850ns) vs one 1.1us exp.
"""

import numpy as np

import concourse.bacc as bacc
import concourse.mybir as mybir
import concourse.tile as tile
from concourse import masks
from concourse.bass_utils import run_bass_kernel_spmd

B, S, D, H = 2, 2048, 1024, 16
HD = D // H            # 64
N_CORES = 8
HPC = H // (N_CORES // B)   # heads per core = 4
HG = HPC * HD               # head-group width = 256

F32 = mybir.dt.float32
F16 = mybir.dt.float16
AF = mybir.ActivationFunctionType
MUL = mybir.AluOpType.mult
P = 128

NDK = D // P     # 8 contraction tiles for projections
NM = HG // P     # 2 row tiles for Q^T/K^T/V^T
NKT = S // P     # 16 key tiles
QC1 = 512        # projection free-dim chunk
QC2 = 1024       # attention q chunk
NQC = S // QC2   # 2

_CACHE = {}


def _build():
    nc = bacc.Bacc("TRN2", target_bir_lowering=False, debug=False,
                   num_devices=N_CORES)

    qt_d = nc.dram_tensor("qt", [D, S], F16, kind="ExternalInput")
    kt_d = nc.dram_tensor("kt", [D, S], F16, kind="ExternalInput")
    vt_d = nc.dram_tensor("vt", [D, S], F16, kind="ExternalInput")
    wq_d = nc.dram_tensor("wq", [D, HG], F16, kind="ExternalInput")
    wk_d = nc.dram_tensor("wk", [D, HG], F16, kind="ExternalInput")
    wv_d = nc.dram_tensor("wv", [D, HG], F16, kind="ExternalInput")
    wo_d = nc.dram_tensor("wo", [HG, D], F16, kind="ExternalInput")
    bq_d = nc.dram_tensor("bq", [HG, 1], F32, kind="ExternalInput")
    bk_d = nc.dram_tensor("bk", [HG, 1], F32, kind="ExternalInput")
    bv_d = nc.dram_tensor("bv", [HG, 1], F32, kind="ExternalInput")
    out_d = nc.dram_tensor("outT", [D, S], F32, kind="ExternalOutput")

    with tile.TileContext(nc) as tc:
        with (
            tc.tile_pool(name="persist", bufs=1) as pp,
            tc.tile_pool(name="aux_ps", bufs=2, space="PSUM") as auxp,
            tc.tile_pool(name="s_ps", bufs=2, space="PSUM") as sps,
            tc.tile_pool(name="o_ps", bufs=1, space="PSUM") as ops,
            tc.tile_pool(name="pt_pool", bufs=24) as ptp,
            tc.tile_pool(name="io_sb", bufs=3) as iop,
            tc.tile_pool(name="sc_sb", bufs=2) as scp,
        ):
            qt_sb = [pp.tile([P, S], F16, tag=f"qt{m}", name=f"qt_sb{m}")
                     for m in range(NM)]
            kt_sb = [pp.tile([P, S], F16, tag=f"kt{m}", name=f"kt_sb{m}")
                     for m in range(NM)]
            at_sb = [pp.tile([P, S], F16, tag=f"at{m}", name=f"at_sb{m}")
                     for m in range(NM)]
            vt_sb = [pp.tile([P, S], F16, tag=f"vt{m}", name=f"vt_sb{m}")
                     for m in range(NM)]
            vb = pp.tile([P, NKT, HPC, 2 * HD], F16)
            wq_sb = pp.tile([P, NDK, HG], F16, tag="wq")
            wk_sb = pp.tile([P, NDK, HG], F16, tag="wk")
            wv_sb = pp.tile([P, NDK, HG], F16, tag="wv")
            wo_sb = pp.tile([P, NM, D], F16, tag="wo")
            bq_sb = pp.tile([P, NM], F32, tag="bq")
            bk_sb = pp.tile([P, NM], F32, tag="bk")
            bv_sb = pp.tile([P, NM], F32, tag="bv")
            ident = pp.tile([P, P], F16)
            ones_t = pp.tile([P, NKT * HPC * HD], F32)

            nc.scalar.dma_start(wq_sb[:], wq_d[:].rearrange("(a p) n -> p a n", p=P))
            nc.scalar.dma_start(wk_sb[:], wk_d[:].rearrange("(a p) n -> p a n", p=P))
            nc.scalar.dma_start(wv_sb[:], wv_d[:].rearrange("(a p) n -> p a n", p=P))
            nc.scalar.dma_start(wo_sb[:], wo_d[:].rearrange("(a p) n -> p a n", p=P))
            nc.scalar.dma_start(bq_sb[:], bq_d[:].rearrange("(a p) o -> p (a o)", p=P))
            nc.scalar.dma_start(bk_sb[:], bk_d[:].rearrange("(a p) o -> p (a o)", p=P))
            nc.scalar.dma_start(bv_sb[:], bv_d[:].rearrange("(a p) o -> p (a o)", p=P))
            masks.make_identity(nc, ident[:])
            nc.gpsimd.memset(ones_t[:], 1.0)
            nc.vector.tensor_copy(
                vb[:, :, :, HD:2 * HD],
                ones_t[:].rearrange("p (a b c) -> p a b c", a=NKT, b=HPC))

            # ---- projections: K^T, V^T first (attention needs them whole),
            # then Q^T. One shared PSUM tag, two slots. ----
            def proj(dram, w_sb, b_sb, dst, dma_eng):
                for qc in range(S // QC1):
                    sl = slice(qc * QC1, (qc + 1) * QC1)
                    xts = []
                    for kt in range(NDK):
                        xt = iop.tile([P, QC1], F16, tag=f"x{kt % 3}",
                                      name=f"x_{kt % 3}")
                        dma_eng.dma_start(xt[:], dram[kt * P:(kt + 1) * P, sl])
                        xts.append(xt)
                    for m in range(NM):
                        ps = auxp.tile([P, QC1], F32, tag="aux", name="proj_ps")
                        cols = slice(m * P, (m + 1) * P)
                        for kt in range(NDK):
                            nc.tensor.matmul(ps[:], w_sb[:, kt, cols], xts[kt][:],
                                             start=(kt == 0), stop=(kt == NDK - 1))
                        nc.vector.tensor_scalar_add(dst[m][:, sl], ps[:],
                                                    b_sb[:, m:m + 1])

            # interleave K and V projections (distinct DMA queues)
            proj(kt_d, wk_sb, bk_sb, kt_sb, nc.sync)
            proj(vt_d, wv_sb, bv_sb, vt_sb, nc.scalar)

            # ---- V^T -> V tiles with ones block ----
            for m in range(NM):
                for f in range(NKT):
                    ptr = sps.tile([P, P], F16, tag="s", name="tr_ps")
                    nc.tensor.transpose(ptr[:], vt_sb[m][:, f * P:(f + 1) * P],
                                        ident[:])
                    nc.vector.tensor_copy(vb[:, f, 2 * m, 0:HD], ptr[:, 0:HD])
                    nc.vector.tensor_copy(vb[:, f, 2 * m + 1, 0:HD],
                                          ptr[:, HD:2 * HD])

            proj(qt_d, wq_sb, bq_sb, qt_sb, nc.sync)

            # ---- attention: head-slots t = 0..7, S-stream of head t
            # interleaved with PV-stream of head t-1 at k-tile grain ----
            def s_kt(qc, h, kt):
                mh, r0 = h // 2, (h % 2) * HD
                hr = slice(r0, r0 + HD)
                kc = slice(kt * P, (kt + 1) * P)
                ps = sps.tile([P, QC2], F32, tag="s", name="s_ps")
                for j in range(QC2 // 512):
                    qj = slice(qc * QC2 + j * 512, qc * QC2 + (j + 1) * 512)
                    nc.tensor.matmul(ps[:, j * 512:(j + 1) * 512],
                                     kt_sb[mh][hr, kc], qt_sb[mh][hr, qj],
                                     start=True, stop=True)
                pt = ptp.tile([P, QC2], F16, tag="pt", name="pt")
                nc.scalar.activation(pt[:], ps[:], AF.Exp)
                return pt

            def pv_kt(h, kt, po, pt):
                for j in range(QC2 // 512):
                    jsl = slice(j * 512, (j + 1) * 512)
                    nc.tensor.matmul(po[:, jsl], vb[:, kt, h, :], pt[:, jsl],
                                     start=(kt == 0), stop=(kt == NKT - 1))

            def finish_head(qc, h, po):
                mh, r0 = h // 2, (h % 2) * HD
                rs = scp.tile([HD, QC2], F32, tag="rs", name="rs")
                nc.vector.tensor_copy(rs[:], po[HD:2 * HD, :])
                rc = scp.tile([HD, QC2], F32, tag="rc", name="rc")
                nc.vector.reciprocal_approx_fast(rc[:], rs[:])
                nc.vector.tensor_tensor(
                    at_sb[mh][r0:r0 + HD, qc * QC2:(qc + 1) * QC2],
                    po[0:HD, :], rc[:], MUL)

            def outproj(qc):
                for m in range(NDK):
                    cols = slice(m * P, (m + 1) * P)
                    for j in range(QC2 // 512):
                        qj = slice(qc * QC2 + j * 512, qc * QC2 + (j + 1) * 512)
                        ps = auxp.tile([P, 512], F32, tag="aux", name="op_ps")
                        for kt2 in range(NM):
                            nc.tensor.matmul(ps[:], wo_sb[:, kt2, cols],
                                             at_sb[kt2][:, qj],
                                             start=(kt2 == 0), stop=(kt2 == NM - 1))
                        ot = iop.tile([P, 512], F32, tag="ot", name="ot")
                        nc.vector.tensor_copy(ot[:], ps[:])
                        nc.sync.dma_start(out_d[cols, qj], ot[:])

            NT = NQC * HPC
            prev = None          # (qc, h, po, pts)
            for t in range(NT):
                qc, h = t // HPC, t % HPC
                po = ops.tile([P, QC2], F32, tag="po", name="po")
                pts = []
                for kt in range(NKT):
                    pts.append(s_kt(qc, h, kt))
                    if prev is not None:
                        pv_kt(prev[1], kt, prev[2], prev[3][kt])
                if prev is not None:
                    finish_head(prev[0], prev[1], prev[2])
                    if prev[0] == 0 and prev[1] == HPC - 1:
                        outproj(0)
                prev = (qc, h, po, pts)
            for kt in range(NKT):
                pv_kt(prev[1], kt, prev[2], prev[3][kt])
            finish_head(prev[0], prev[1], prev[2])
            outproj(1)

    nc.compile()
    return nc


def kernel(query, key, value, Wq, bq, Wk, bk, Wv, bv, Wo, bo):
    if "nc" not in _CACHE:
        _CACHE["nc"] = _build()
    nc = _CACHE["nc"]

    scale = np.float32(1.0 / np.sqrt(HD))
    xt = {}
    for b in range(B):
        xt[("q", b)] = np.ascontiguousarray(query[b].T).astype(np.float16)
        xt[("k", b)] = np.ascontiguousarray(key[b].T).astype(np.float16)
        xt[("v", b)] = np.ascontiguousarray(value[b].T).astype(np.float16)

    in_maps = []
    for c in range(N_CORES):
        b, g = c // (N_CORES // B), c % (N_CORES // B)
        cols = slice(g * HG, (g + 1) * HG)
        in_maps.append({
            "qt": xt[("q", b)],
            "kt": xt[("k", b)],
            "vt": xt[("v", b)],
            "wq": (np.ascontiguousarray(Wq[:, cols]) * scale).astype(np.float16),
            "wk": np.ascontiguousarray(Wk[:, cols]).astype(np.float16),
            "wv": np.ascontiguousarray(Wv[:, cols]).astype(np.float16),
            "wo": np.ascontiguousarray(Wo[cols, :]).astype(np.float16),
            "bq": (bq[cols] * scale).reshape(HG, 1).astype(np.float32),
            "bk": bk[cols].reshape(HG, 1).astype(np.float32),
            "bv": bv[cols].reshape(HG, 1).astype(np.float32),
        })

    global _last_in_maps
    _last_in_maps = in_maps
    res = run_bass_kernel_spmd(nc, in_maps, list(range(N_CORES)))

    out = np.zeros((B, S, D), dtype=np.float32)
    for c in range(N_CORES):
        b = c // (N_CORES // B)
        out[b] += res.results[c]["outT"].T
    out += bo.astype(np.float32)
    return out


# revision 9
# speedup vs baseline: 1.8691x; 1.1360x over previous
"""Multi-head attention (B=2, S=2048, D=1024, H=16) on 8 TRN2 NeuronCores.

Sharding: core c handles batch b = c // 4 and heads 4*(c%4) .. 4*(c%4)+3.
Each core computes its 4 heads' Q/K/V projections (column slices of the
weights), head-local attention, and a partial output projection (row slice
of Wo). Host sums the 4 partials per batch. No cross-device collectives.

Layout: everything transposed. Inputs arrive as x^T [D, S] fp16, so the
projections produce Q^T/K^T directly (lhsT = weight slice, rhs = x^T).
Scores are computed per head as S^T = K_h^T.T @ Q_h^T in [k, q] layout, exp
runs on ScalarE straight out of PSUM into fp16 staging tiles, and the P@V
matmul uses V tiles augmented with a 64-wide ones block so the softmax
denominators appear as 64 replicated rows of the same PSUM accumulator
(full-width DVE reciprocal, no partition broadcast). The output projection
emits the partial out^T [D, S] in fp32; the host transposes and sums.

The attention streams for head t and head t-1 are interleaved at k-tile
granularity so the tensor engine never waits on the (slower) ScalarE exp
stream: per k-tile the PE does 4 matmuls (~850ns) vs one 1.1us exp.
"""

import numpy as np

import concourse.bacc as bacc
import concourse.mybir as mybir
import concourse.tile as tile
from concourse import masks
from concourse.bass_utils import run_bass_kernel_spmd

B, S, D, H = 2, 2048, 1024, 16
HD = D // H            # 64
N_CORES = 8
HPC = H // (N_CORES // B)   # heads per core = 4
HG = HPC * HD               # head-group width = 256

F32 = mybir.dt.float32
F16 = mybir.dt.float16
AF = mybir.ActivationFunctionType
MUL = mybir.AluOpType.mult
P = 128

NDK = D // P     # 8 contraction tiles for projections
NM = HG // P     # 2 row tiles for Q^T/K^T/V^T
NKT = S // P     # 16 key tiles
QC1 = 512        # projection free-dim chunk
QC2 = 1024       # attention q chunk
NQC = S // QC2   # 2

_CACHE = {}


def _build():
    nc = bacc.Bacc("TRN2", target_bir_lowering=False, debug=False,
                   num_devices=N_CORES)

    qt_d = nc.dram_tensor("qt", [D, S], F16, kind="ExternalInput")
    kt_d = nc.dram_tensor("kt", [D, S], F16, kind="ExternalInput")
    vt_d = nc.dram_tensor("vt", [D, S], F16, kind="ExternalInput")
    wq_d = nc.dram_tensor("wq", [D, HG], F16, kind="ExternalInput")
    wk_d = nc.dram_tensor("wk", [D, HG], F16, kind="ExternalInput")
    wv_d = nc.dram_tensor("wv", [D, HG], F16, kind="ExternalInput")
    wo_d = nc.dram_tensor("wo", [HG, D], F16, kind="ExternalInput")
    bq_d = nc.dram_tensor("bq", [HG, 1], F32, kind="ExternalInput")
    bk_d = nc.dram_tensor("bk", [HG, 1], F32, kind="ExternalInput")
    bv_d = nc.dram_tensor("bv", [HG, 1], F32, kind="ExternalInput")
    out_d = nc.dram_tensor("outT", [D, S], F32, kind="ExternalOutput")

    with tile.TileContext(nc) as tc:
        with (
            tc.tile_pool(name="persist", bufs=1) as pp,
            tc.tile_pool(name="s_ps", bufs=2, space="PSUM") as sps,
            tc.tile_pool(name="big_ps", bufs=2, space="PSUM") as bigp,
            tc.tile_pool(name="pt_pool", bufs=24) as ptp,
            tc.tile_pool(name="io_sb", bufs=3) as iop,
            tc.tile_pool(name="sc_sb", bufs=2) as scp,
        ):
            qt_sb = [pp.tile([P, S], F16, tag=f"qt{m}", name=f"qt_sb{m}")
                     for m in range(NM)]
            kt_sb = [pp.tile([P, S], F16, tag=f"kt{m}", name=f"kt_sb{m}")
                     for m in range(NM)]
            at_sb = [pp.tile([P, S], F16, tag=f"at{m}", name=f"at_sb{m}")
                     for m in range(NM)]
            vt_sb = [pp.tile([P, S], F16, tag=f"vt{m}", name=f"vt_sb{m}")
                     for m in range(NM)]
            vb = pp.tile([P, NKT, HPC, 2 * HD], F16)
            wq_sb = pp.tile([P, NDK, HG], F16, tag="wq")
            wk_sb = pp.tile([P, NDK, HG], F16, tag="wk")
            wv_sb = pp.tile([P, NDK, HG], F16, tag="wv")
            wo_sb = pp.tile([P, NM, D], F16, tag="wo")
            bq_sb = pp.tile([P, NM], F32, tag="bq")
            bk_sb = pp.tile([P, NM], F32, tag="bk")
            bv_sb = pp.tile([P, NM], F32, tag="bv")
            ident = pp.tile([P, P], F16)
            ones_t = pp.tile([P, NKT * HPC * HD], F32)

            nc.scalar.dma_start(wq_sb[:], wq_d[:].rearrange("(a p) n -> p a n", p=P))
            nc.scalar.dma_start(wk_sb[:], wk_d[:].rearrange("(a p) n -> p a n", p=P))
            nc.scalar.dma_start(wv_sb[:], wv_d[:].rearrange("(a p) n -> p a n", p=P))
            nc.scalar.dma_start(wo_sb[:], wo_d[:].rearrange("(a p) n -> p a n", p=P))
            nc.scalar.dma_start(bq_sb[:], bq_d[:].rearrange("(a p) o -> p (a o)", p=P))
            nc.scalar.dma_start(bk_sb[:], bk_d[:].rearrange("(a p) o -> p (a o)", p=P))
            nc.scalar.dma_start(bv_sb[:], bv_d[:].rearrange("(a p) o -> p (a o)", p=P))
            masks.make_identity(nc, ident[:])
            nc.gpsimd.memset(ones_t[:], 1.0)
            nc.vector.tensor_copy(
                vb[:, :, :, HD:2 * HD],
                ones_t[:].rearrange("p (a b c) -> p a b c", a=NKT, b=HPC))

            # ---- projections: K^T, V^T first (attention needs them whole),
            # then Q^T. One shared PSUM tag, two slots. ----
            def proj(dram, w_sb, b_sb, dst, unused=None):
                for qc in range(S // QC1):
                    sl = slice(qc * QC1, (qc + 1) * QC1)
                    xts = []
                    for kt in range(NDK):
                        xt = iop.tile([P, QC1], F16, tag=f"x{kt % 3}",
                                      name=f"x_{kt % 3}")
                        eng = nc.sync if kt % 2 == 0 else nc.scalar
                        eng.dma_start(xt[:], dram[kt * P:(kt + 1) * P, sl])
                        xts.append(xt)
                    for m in range(NM):
                        ps = bigp.tile([P, QC1], F32, tag="big", name="proj_ps", padded_shape=[P, QC2])
                        cols = slice(m * P, (m + 1) * P)
                        for kt in range(NDK):
                            nc.tensor.matmul(ps[:], w_sb[:, kt, cols], xts[kt][:],
                                             start=(kt == 0), stop=(kt == NDK - 1))
                        nc.vector.tensor_scalar_add(dst[m][:, sl], ps[:],
                                                    b_sb[:, m:m + 1])

            # interleave K and V projections (distinct DMA queues)
            proj(kt_d, wk_sb, bk_sb, kt_sb)
            proj(vt_d, wv_sb, bv_sb, vt_sb)

            # ---- V^T -> V tiles with ones block ----
            for m in range(NM):
                for f in range(NKT):
                    ptr = sps.tile([P, P], F16, tag="s", name="tr_ps")
                    nc.tensor.transpose(ptr[:], vt_sb[m][:, f * P:(f + 1) * P],
                                        ident[:])
                    nc.vector.tensor_copy(vb[:, f, 2 * m, 0:HD], ptr[:, 0:HD])
                    nc.vector.tensor_copy(vb[:, f, 2 * m + 1, 0:HD],
                                          ptr[:, HD:2 * HD])

            proj(qt_d, wq_sb, bq_sb, qt_sb)

            # ---- attention: head-slots t = 0..7, S-stream of head t
            # interleaved with PV-stream of head t-1 at k-tile grain ----
            def s_kt(qc, h, kt):
                mh, r0 = h // 2, (h % 2) * HD
                hr = slice(r0, r0 + HD)
                kc = slice(kt * P, (kt + 1) * P)
                ps = sps.tile([P, QC2], F32, tag="s", name="s_ps")
                for j in range(QC2 // 512):
                    qj = slice(qc * QC2 + j * 512, qc * QC2 + (j + 1) * 512)
                    nc.tensor.matmul(ps[:, j * 512:(j + 1) * 512],
                                     kt_sb[mh][hr, kc], qt_sb[mh][hr, qj],
                                     start=True, stop=True)
                pt = ptp.tile([P, QC2], F16, tag="pt", name="pt")
                nc.scalar.activation(pt[:], ps[:], AF.Exp)
                return pt

            def pv_kt(h, kt, po, pt):
                for j in range(QC2 // 512):
                    jsl = slice(j * 512, (j + 1) * 512)
                    nc.tensor.matmul(po[:, jsl], vb[:, kt, h, :], pt[:, jsl],
                                     start=(kt == 0), stop=(kt == NKT - 1))

            def finish_head(qc, h, po):
                mh, r0 = h // 2, (h % 2) * HD
                rs = scp.tile([HD, QC2], F32, tag="rs", name="rs")
                nc.vector.tensor_copy(rs[:], po[HD:2 * HD, :])
                rc = scp.tile([HD, QC2], F32, tag="rc", name="rc")
                nc.vector.reciprocal_approx_fast(rc[:], rs[:])
                nc.vector.tensor_tensor(
                    at_sb[mh][r0:r0 + HD, qc * QC2:(qc + 1) * QC2],
                    po[0:HD, :], rc[:], MUL)

            def outproj(qc):
                for m in range(NDK):
                    cols = slice(m * P, (m + 1) * P)
                    for j in range(QC2 // 512):
                        qj = slice(qc * QC2 + j * 512, qc * QC2 + (j + 1) * 512)
                        ps = bigp.tile([P, 512], F32, tag="big", name="op_ps", padded_shape=[P, QC2])
                        for kt2 in range(NM):
                            nc.tensor.matmul(ps[:], wo_sb[:, kt2, cols],
                                             at_sb[kt2][:, qj],
                                             start=(kt2 == 0), stop=(kt2 == NM - 1))
                        ot = iop.tile([P, 512], F32, tag="ot", name="ot")
                        nc.vector.tensor_copy(ot[:], ps[:])
                        eng = nc.sync if (m + j) % 2 == 0 else nc.scalar
                        eng.dma_start(out_d[cols, qj], ot[:])

            NT = NQC * HPC
            prev = None          # (qc, h, po, pts)
            for t in range(NT):
                qc, h = t // HPC, t % HPC
                po = bigp.tile([P, QC2], F32, tag="big", name="po")
                pts = []
                for kt in range(NKT):
                    pts.append(s_kt(qc, h, kt))
                    if prev is not None:
                        pv_kt(prev[1], kt, prev[2], prev[3][kt])
                if prev is not None:
                    finish_head(prev[0], prev[1], prev[2])
                    if prev[0] == 0 and prev[1] == HPC - 1:
                        outproj(0)
                prev = (qc, h, po, pts)
            for kt in range(NKT):
                pv_kt(prev[1], kt, prev[2], prev[3][kt])
            finish_head(prev[0], prev[1], prev[2])
            outproj(1)

    nc.compile()
    return nc


def kernel(query, key, value, Wq, bq, Wk, bk, Wv, bv, Wo, bo):
    if "nc" not in _CACHE:
        _CACHE["nc"] = _build()
    nc = _CACHE["nc"]

    scale = np.float32(1.0 / np.sqrt(HD))
    xt = {}
    for b in range(B):
        xt[("q", b)] = np.ascontiguousarray(query[b].T).astype(np.float16)
        xt[("k", b)] = np.ascontiguousarray(key[b].T).astype(np.float16)
        xt[("v", b)] = np.ascontiguousarray(value[b].T).astype(np.float16)

    in_maps = []
    for c in range(N_CORES):
        b, g = c // (N_CORES // B), c % (N_CORES // B)
        cols = slice(g * HG, (g + 1) * HG)
        in_maps.append({
            "qt": xt[("q", b)],
            "kt": xt[("k", b)],
            "vt": xt[("v", b)],
            "wq": (np.ascontiguousarray(Wq[:, cols]) * scale).astype(np.float16),
            "wk": np.ascontiguousarray(Wk[:, cols]).astype(np.float16),
            "wv": np.ascontiguousarray(Wv[:, cols]).astype(np.float16),
            "wo": np.ascontiguousarray(Wo[cols, :]).astype(np.float16),
            "bq": (bq[cols] * scale).reshape(HG, 1).astype(np.float32),
            "bk": bk[cols].reshape(HG, 1).astype(np.float32),
            "bv": bv[cols].reshape(HG, 1).astype(np.float32),
        })

    global _last_in_maps
    _last_in_maps = in_maps
    res = run_bass_kernel_spmd(nc, in_maps, list(range(N_CORES)))

    out = np.zeros((B, S, D), dtype=np.float32)
    for c in range(N_CORES):
        b = c // (N_CORES // B)
        out[b] += res.results[c]["outT"].T
    out += bo.astype(np.float32)
    return out
